# revision 35
# baseline (speedup 1.0000x reference)
"""Trainium2 Bass kernel for additive (Bahdanau) attention, data-parallel over batch.

Reference computation (B=32, L=2048, D=1024):
    q = query[:,0,:] @ W_in.T                        # [B,D]
    scores = tanh(q@w_lo + context@w_hi + b_att)     # [B,L]
    scores = where(mask==0, -1e9, scores)
    weights = softmax(scores, -1)                    # [B,L]
    mix = weights @ context                          # [B,D]
    out = tanh([mix, q] @ W_out.T)                   # [B,D]
    returns (out[:,None,:], weights[:,None,:])

Sharding: 8 cores x 4 batches each; weights replicated. All FLOPs on device;
host only re-lays-out inputs (transpose/broadcast/dtype prep) and concatenates
per-core outputs.

Per-core dataflow: context[b] streams as [128p, 16t, 1024d] quarters
(l = p*16 + t). Scores: VectorE multiply + ScalarE Copy-accumulate row sums.
Softmax: ScalarE tanh/exp (exp with fused per-partition sum), PE ones-matmul
partition reduce, VectorE reciprocal. Mix: PE accumulation chain, weight
column stationary, context moving (N=512). Final [mix,q] @ W_out^T fused
across the 4 batches.
"""

import sys

import numpy as np

if "/opt/trn_rl_repo" not in sys.path:
    sys.path.insert(0, "/opt/trn_rl_repo")

import concourse.bass as bass
import concourse.bass_utils as bass_utils
import concourse.tile as tile
from concourse import mybir
from concourse.bass_utils import run_bass_kernel_spmd
from concourse.masks import make_identity

B, L, D = 32, 2048, 1024
NCORES = 8
BPC = B // NCORES  # batches per core
P = 128            # partitions
T = L // P         # 16 free-dim L-tiles per batch; l = p*T + t
NQ = 4             # context quarters per batch
QT = T // NQ       # L-tiles per quarter
KD = D // P        # 8 contraction chunks over D
KJ = 2 * D // P    # 16 contraction chunks over 2D
DT = mybir.dt.float32
F32 = np.float32
AF = mybir.ActivationFunctionType

# ---------------------------------------------------------------------------
# Compiler workarounds for the walrus build in this container.
#
# 1. It rejects instructions carrying more than one sync-wait ("Too many sync
#    wait commands"). Tile freely attaches several waits per instruction, so
#    after scheduling we hoist all but the last wait of each instruction onto
#    single-wait nops inserted just before it on the same engine (waits are
#    AND conditions, so splitting across same-engine predecessors is
#    equivalent).
# 2. bir_verify_and_optimise hardcodes --enable-ldw-opt=false, which
#    serializes every LDWEIGHTS with its MATMUL; flip it to true.
# ---------------------------------------------------------------------------
_MAX_WAITS = 1


def _split_multi_waits(nc: bass.Bass) -> None:
    uid = 0
    for fn in nc.m.functions:
        for blk in fn.blocks:
            insts = blk.instructions
            if not any(
                i.sync_info is not None and len(i.sync_info.on_wait) > _MAX_WAITS
                for i in insts
            ):
                continue
            out = []
            for inst in insts:
                si = inst.sync_info
                if si is not None and len(si.on_wait) > _MAX_WAITS:
                    waits = list(si.on_wait)
                    head, tail = waits[:-_MAX_WAITS], waits[-_MAX_WAITS:]
                    for i in range(0, len(head), _MAX_WAITS):
                        nop = mybir.InstNoOp(
                            name=f"{inst.name}-wsplit{uid}", ins=[], outs=[]
                        )
                        uid += 1
                        nop.engine = inst.engine
                        nop.sync_info = mybir.SyncInfo(
                            on_wait=head[i : i + _MAX_WAITS], on_update=[]
                        )
                        out.append(nop)
                    inst.sync_info = mybir.SyncInfo(
                        on_wait=tail, on_update=list(si.on_update)
                    )
                out.append(inst)
            insts[:] = out


_LDW_PATCHED = False


def _enable_ldw_opt() -> None:
    global _LDW_PATCHED
    if _LDW_PATCHED:
        return
    _LDW_PATCHED = True
    for mod_name in ("concourse.bass_utils",):
        mod = sys.modules.get(mod_name)
        if mod is None:
            continue
        orig_run = mod.run_command

        def patched_run(cmd, *a, _orig=orig_run, **k):
            cmd = [
                c.replace("--enable-ldw-opt=false", "--enable-ldw-opt=true")
                if isinstance(c, str)
                else c
                for c in cmd
            ]
            return _orig(cmd, *a, **k)

        mod.run_command = patched_run


def build_graph() -> bass.Bass:
    nc = bass.Bass()

    ctx_d = nc.declare_dram_parameter("ctx", [BPC, L, D], DT, isOutput=False)
    maskb_d = nc.declare_dram_parameter("maskb", [BPC, L], DT, isOutput=False)
    qT_d = nc.declare_dram_parameter("qT", [P, KD, BPC], DT, isOutput=False)
    winT_d = nc.declare_dram_parameter("winT", [D, D], DT, isOutput=False)
    woutT_d = nc.declare_dram_parameter("woutT", [2 * D, D], DT, isOutput=False)
    whib_d = nc.declare_dram_parameter("whib", [P, D], DT, isOutput=False)
    wlo4_d = nc.declare_dram_parameter("wlo4", [BPC, D], DT, isOutput=False)
    batt4_d = nc.declare_dram_parameter("batt4", [BPC, 1], DT, isOutput=False)
    out_d = nc.declare_dram_parameter("out", [BPC, D], DT, isOutput=True)
    wts_d = nc.declare_dram_parameter("wts", [BPC, L], DT, isOutput=True)

    with tile.TileContext(nc) as tc:
        with (
            tc.tile_pool(name="singles", bufs=1) as singles,
            tc.tile_pool(name="ctxp", bufs=10) as ctxp,
            tc.tile_pool(name="scrp", bufs=2) as scrp,
            tc.tile_pool(name="smp", bufs=2) as smp,
            tc.tile_pool(name="qout", bufs=1) as qout,
            tc.tile_pool(name="ps_big", bufs=1, space="PSUM") as ps_big,
            tc.tile_pool(name="ps_mix", bufs=1, space="PSUM") as ps_mix,
            tc.tile_pool(name="ps_col", bufs=2, space="PSUM") as ps_col,
            tc.tile_pool(name="ps_tiny", bufs=2, space="PSUM") as ps_tiny,
        ):
            # ---- small constants ----
            whib = singles.tile([P, D], DT)
            nc.gpsimd.dma_start(out=whib, in_=whib_d[:, :])
            wlo4 = singles.tile([BPC, D], DT)
            nc.gpsimd.dma_start(out=wlo4, in_=wlo4_d[:, :])
            batt4 = singles.tile([BPC, 1], DT)
            nc.gpsimd.dma_start(out=batt4, in_=batt4_d[:, :])
            qTs = singles.tile([P, KD, BPC], DT)
            nc.gpsimd.dma_start(out=qTs, in_=qT_d[:, :, :])
            ones_col = singles.tile([P, 1], DT)
            nc.vector.memset(ones_col, 1.0)
            ones_row = singles.tile([1, P], DT)
            nc.vector.memset(ones_row, 1.0)
            ident = singles.tile([32, 32], DT)
            make_identity(nc, ident)
            # combined^T: j-chunk k, 4 batch columns. j<D: mix part; j>=D: q.
            comb = singles.tile([P, KJ, BPC], DT)
            m4_sb = qout.tile([BPC, D], DT, tag="m4")  # mix rows, 4 batches

            def emit_ctx(b):
                """Context DMAs (one per L-tile for transfer concurrency —
                a single transfer tops out ~25 GB/s; aggregate bandwidth
                comes from many concurrent transfers)."""
                ctx_re = ctx_d[b].rearrange("(p t) d -> p t d", p=P)
                quarters = []
                for qd in range(NQ):
                    hq = ctxp.tile([P, QT, D], DT, tag="ctx")
                    for j in range(QT):
                        nc.sync.dma_start(
                            out=hq[:, j, :], in_=ctx_re[:, qd * QT + j, :]
                        )
                    quarters.append(hq)
                mb = smp.tile([P, T], DT, tag="mb")
                nc.gpsimd.dma_start(
                    out=mb, in_=maskb_d[b].rearrange("(p t) -> p t", p=P)
                )
                s_all = smp.tile([P, T], DT, tag="sall")
                return quarters, mb, s_all

            def emit_score_quarter(quarters, s_all, qd):
                for tt in range(QT):
                    t = qd * QT + tt
                    scr_t = scrp.tile([P, D], DT, tag="scr")
                    nc.vector.tensor_mul(scr_t, quarters[qd][:, tt, :], whib)
                    nc.scalar.activation(
                        scr_t, scr_t, AF.Copy, accum_out=s_all[:, t : t + 1]
                    )

            # ---- q = query @ W_in^T ----
            # W_in streamed through two context-pool slots (freed after the
            # q matmuls); all 8 DMAs issued at once for transfer concurrency.
            win_a = ctxp.tile([P, QT, D], DT, tag="ctx")
            win_b = ctxp.tile([P, QT, D], DT, tag="ctx")
            for k in range(KD):
                wt = (win_a, win_b)[k // QT]
                nc.gpsimd.dma_start(
                    out=wt[:, k % QT, :], in_=winT_d[k * P : (k + 1) * P, :]
                )
            q_ps = ps_big.tile([BPC, D], DT, tag="big")
            for k in range(KD):
                wt = (win_a, win_b)[k // QT]
                for n2 in range(2):
                    nc.tensor.matmul(
                        q_ps[:, n2 * 512 : (n2 + 1) * 512],
                        lhsT=qTs[:, k, :],
                        rhs=wt[:, k % QT, n2 * 512 : (n2 + 1) * 512],
                        start=(k == 0),
                        stop=(k == KD - 1),
                    )

            # batch-0 context + first two score quarters ahead of the q tail
            state0 = emit_ctx(0)
            emit_score_quarter(state0[0], state0[2], 0)
            emit_score_quarter(state0[0], state0[2], 1)

            # bias chain: q_part = q . w_lo (VectorE reads q straight from
            # PSUM), + b_att, transposed to a row, broadcast across partitions
            qp = smp.tile([BPC, 1], DT, tag="qp")
            scr_q = scrp.tile([BPC, D], DT, tag="scr")
            nc.vector.tensor_mul(scr_q, q_ps, wlo4)
            nc.scalar.activation(scr_q, scr_q, AF.Copy, accum_out=qp)
            bias4 = smp.tile([BPC, 1], DT, tag="bias4")
            nc.vector.tensor_add(bias4, qp, batt4)
            brow_ps = ps_tiny.tile([1, BPC], DT, tag="tiny")
            nc.tensor.transpose(brow_ps, bias4, ident[:BPC, :BPC])
            brow = smp.tile([1, BPC], DT, tag="brow")
            nc.scalar.copy(brow, brow_ps)
            bbc_ps = ps_col.tile([P, BPC], DT, tag="col")
            nc.tensor.matmul(bbc_ps, lhsT=ones_row, rhs=brow, start=True, stop=True)
            bias_bc = smp.tile([P, BPC], DT, tag="bbc")
            nc.scalar.copy(bias_bc, bbc_ps)

            # ---- batch pipeline: scores, softmax and mix interleaved at
            # quarter granularity. tanh/exp are elementwise and the mix chain
            # uses UNNORMALIZED exp weights (1/Z folded into the copy-out
            # scale), so each quarter's mix matmuls fire as soon as its
            # scores exist — no end-of-batch serial mix chain.
            woutT_re = woutT_d[:, :].rearrange("(k p) e -> p k e", p=P)
            prev_mix = None  # (mix_ps, invz, batch) pending scale + copy-out
            wout_supers = []
            for b in range(BPC):
                quarters, mb, s_all = state0 if b == 0 else emit_ctx(b)

                if b == BPC - 1:
                    # W_out^T streamed through freed context slots: its DMAs
                    # start while this batch computes, hiding the 8MB load.
                    for s in range(4):
                        wsup = ctxp.tile([P, QT, D], DT, tag="ctx")
                        for j in range(QT):
                            nc.sync.dma_start(
                                out=wsup[:, j, :], in_=woutT_re[:, 4 * s + j, :]
                            )
                        wout_supers.append(wsup)

                # previous batch's mix: scale by 1/Z on ScalarE, DMA into its
                # m4 row (engines cannot write at a partition offset; DMA can)
                if prev_mix is not None:
                    pm, pz, pb = prev_mix
                    mix_sb = scrp.tile([1, D], DT, tag="scr")
                    nc.scalar.mul(mix_sb, pm, pz)
                    nc.gpsimd.dma_start(out=m4_sb[pb : pb + 1, :], in_=mix_sb)
                    prev_mix = None

                s_tanh = smp.tile([P, T], DT, tag="stanh")
                s_m = smp.tile([P, T], DT, tag="sm")
                p_exp = smp.tile([P, T], DT, tag="pexp")
                zq = smp.tile([P, NQ], DT, tag="zq")
                mix_ps = ps_mix.tile([1, D], DT, tag="mixrow")
                for qd in range(NQ):
                    if not (b == 0 and qd < 2):
                        emit_score_quarter(quarters, s_all, qd)
                    cs = slice(qd * QT, (qd + 1) * QT)
                    nc.scalar.activation(
                        s_tanh[:, cs],
                        s_all[:, cs],
                        AF.Tanh,
                        bias=bias_bc[:, b : b + 1],
                        scale=1.0,
                    )
                    nc.vector.tensor_add(s_m[:, cs], s_tanh[:, cs], mb[:, cs])
                    nc.scalar.activation(
                        p_exp[:, cs],
                        s_m[:, cs],
                        AF.Exp,
                        accum_out=zq[:, qd : qd + 1],
                    )
                    for tt in range(QT):
                        t = qd * QT + tt
                        for n2 in range(2):
                            nc.tensor.matmul(
                                mix_ps[:, n2 * 512 : (n2 + 1) * 512],
                                lhsT=p_exp[:, t : t + 1],
                                rhs=quarters[qd][
                                    :, tt, n2 * 512 : (n2 + 1) * 512
                                ],
                                start=(t == 0),
                                stop=(t == T - 1),
                            )

                # normalizer + weights output (off the mix critical path)
                zcol = smp.tile([P, 1], DT, tag="zcol")
                nc.vector.tensor_reduce(
                    zcol, zq, axis=mybir.AxisListType.X, op=mybir.AluOpType.add
                )
                z_ps = ps_tiny.tile([1, 1], DT, tag="tiny")
                nc.tensor.matmul(z_ps, lhsT=ones_col, rhs=zcol, start=True, stop=True)
                invz = smp.tile([1, 1], DT, tag="invz")
                nc.vector.reciprocal(invz, z_ps)
                izb_ps = ps_col.tile([P, 1], DT, tag="col")
                nc.tensor.matmul(
                    izb_ps, lhsT=ones_row, rhs=invz, start=True, stop=True
                )
                izb = smp.tile([P, 1], DT, tag="izb")
                nc.scalar.copy(izb, izb_ps)
                w_sb = smp.tile([P, T], DT, tag="wsb")
                nc.scalar.mul(w_sb, p_exp, izb)
                nc.gpsimd.dma_start(
                    out=wts_d[b].rearrange("(p t) -> p t", p=P), in_=w_sb
                )
                prev_mix = (mix_ps, invz, b)

            # Last batch's deferred outputs must hit the sync stream BEFORE
            # the W_out^T stream: its tile DMAs are slot-gated on the final
            # matmuls, which transitively need the m4 DMA below.
            pm, pz, pb = prev_mix
            mix_sb = scrp.tile([1, D], DT, tag="scr")
            nc.scalar.mul(mix_sb, pm, pz)
            nc.gpsimd.dma_start(out=m4_sb[pb : pb + 1, :], in_=mix_sb)

            # q rows -> comb columns (j >= D half), via PE transpose
            q_sb = qout.tile([BPC, D], DT, tag="qbuf")
            nc.scalar.copy(q_sb, q_ps)
            for k in range(KD):
                qt_ps = ps_col.tile([P, BPC], DT, tag="col")
                nc.tensor.transpose(
                    qt_ps, q_sb[:, k * P : (k + 1) * P], ident[:BPC, :BPC]
                )
                nc.vector.tensor_copy(comb[:, KD + k, :], qt_ps)

            # mix rows -> comb columns (j < D half), all 4 batches at once
            for k in range(KD):
                mt_ps = ps_col.tile([P, BPC], DT, tag="col")
                nc.tensor.transpose(
                    mt_ps, m4_sb[:, k * P : (k + 1) * P], ident[:BPC, :BPC]
                )
                nc.vector.tensor_copy(comb[:, k, :], mt_ps)

            # ---- out = tanh(combined @ W_out^T), fused over all 4 batches ----
            out_ps = ps_big.tile([BPC, D], DT, tag="big")
            for k in range(KJ):
                for n2 in range(2):
                    nc.tensor.matmul(
                        out_ps[:, n2 * 512 : (n2 + 1) * 512],
                        lhsT=comb[:, k, :],
                        rhs=wout_supers[k // QT][:, k % QT, n2 * 512 : (n2 + 1) * 512],
                        start=(k == 0),
                        stop=(k == KJ - 1),
                    )
            out_sb = qout.tile([BPC, D], DT, tag="qbuf")
            nc.scalar.activation(out_sb, out_ps, AF.Tanh)
            nc.sync.dma_start(out=out_d[:, :], in_=out_sb)

    _split_multi_waits(nc)
    return nc


_GRAPH = None


def _get_graph() -> bass.Bass:
    global _GRAPH
    if _GRAPH is None:
        _GRAPH = build_graph()
    return _GRAPH


def _prep_in_maps(inputs: dict) -> list[dict]:
    query = np.asarray(inputs["query"], dtype=F32)      # [B,1,D]
    context = np.asarray(inputs["context"], dtype=F32)  # [B,L,D]
    W_in = np.asarray(inputs["W_in"], dtype=F32)        # [D,D]
    w_att = np.asarray(inputs["w_att"], dtype=F32)      # [2D]
    b_att = np.asarray(inputs["b_att"], dtype=F32)      # [1]
    W_out = np.asarray(inputs["W_out"], dtype=F32)      # [D,2D]
    mask = np.asarray(inputs["mask"])                   # [B,L] int32

    winT = np.ascontiguousarray(W_in.T)
    woutT = np.ascontiguousarray(W_out.T)
    whib = np.ascontiguousarray(np.broadcast_to(w_att[D:], (P, D)))
    wlo4 = np.ascontiguousarray(np.broadcast_to(w_att[:D], (BPC, D)))
    batt4 = np.full((BPC, 1), float(b_att[0]), dtype=F32)
    maskb = (mask.astype(F32) - 1.0) * 1e9              # 0 kept, -1e9 masked

    in_maps = []
    for c in range(NCORES):
        bs = slice(c * BPC, (c + 1) * BPC)
        q_c = query[bs, 0, :]                           # [BPC, D]
        qT_c = np.ascontiguousarray(
            np.ascontiguousarray(q_c.T).reshape(KD, P, BPC).transpose(1, 0, 2)
        )                                               # [P, KD, BPC]
        in_maps.append(
            {
                "ctx": np.ascontiguousarray(context[bs]),
                "maskb": np.ascontiguousarray(maskb[bs]),
                "qT": qT_c,
                "winT": winT,
                "woutT": woutT,
                "whib": whib,
                "wlo4": wlo4,
                "batt4": batt4,
            }
        )
    return in_maps


def _run(inputs: dict, trace: bool = False):
    _enable_ldw_opt()
    nc = _get_graph()
    in_maps = _prep_in_maps(inputs)
    res = run_bass_kernel_spmd(nc, in_maps, core_ids=list(range(NCORES)), trace=trace)
    out = np.concatenate([r["out"] for r in res.results], axis=0)[:, None, :]
    wts = np.concatenate([r["wts"] for r in res.results], axis=0)[:, None, :]
    return (out.astype(F32), wts.astype(F32)), res


def kernel(**inputs):
    (out, wts), _ = _run(inputs)
    return out, wts


# revision 36
# speedup vs baseline: 1.3208x; 1.3208x over previous
"""Trainium2 Bass kernel for additive (Bahdanau) attention, data-parallel over batch.

Reference computation (B=32, L=2048, D=1024):
    q = query[:,0,:] @ W_in.T                        # [B,D]
    scores = tanh(q@w_lo + context@w_hi + b_att)     # [B,L]
    scores = where(mask==0, -1e9, scores)
    weights = softmax(scores, -1)                    # [B,L]
    mix = weights @ context                          # [B,D]
    out = tanh([mix, q] @ W_out.T)                   # [B,D]
    returns (out[:,None,:], weights[:,None,:])

Sharding: 8 cores x 4 batches each; weights replicated. All FLOPs on device;
host only re-lays-out inputs (transpose/broadcast/dtype prep) and concatenates
per-core outputs.

Per-core dataflow: context[b] streams as [128p, 16t, 1024d] quarters
(l = p*16 + t), one dma_start per L-tile (single transfers top out ~25 GB/s;
aggregate bandwidth needs many concurrent transfers). Scores: VectorE multiply
+ ScalarE Copy-accumulate row sums. Softmax: ScalarE tanh/exp (exp with fused
per-partition sum), PE ones-matmul partition reduce, VectorE reciprocal. Mix:
PE accumulation chain with UNNORMALIZED exp weights (1/Z folded into the
copy-out scale), exp-weight column stationary, context moving (N=512). Final
[mix,q] @ W_out^T fused across the 4 batches, W_out^T streamed through freed
context-pool slots.
"""

import sys

import numpy as np

if "/opt/trn_rl_repo" not in sys.path:
    sys.path.insert(0, "/opt/trn_rl_repo")

import concourse.bass as bass
import concourse.tile as tile
from concourse import mybir
from concourse.bass_utils import run_bass_kernel_spmd
from concourse.masks import make_identity

B, L, D = 32, 2048, 1024
NCORES = 8
BPC = B // NCORES  # batches per core
P = 128            # partitions
T = L // P         # 16 free-dim L-tiles per batch; l = p*T + t
NQ = 4             # context quarters per batch
QT = T // NQ       # L-tiles per quarter
KD = D // P        # 8 contraction chunks over D
KJ = 2 * D // P    # 16 contraction chunks over 2D
DT = mybir.dt.float32
F32 = np.float32
AF = mybir.ActivationFunctionType

# ---------------------------------------------------------------------------
# Compiler workarounds for the walrus build in this container.
#
# 1. It rejects instructions carrying more than one sync-wait ("Too many sync
#    wait commands"). Tile freely attaches several waits per instruction, so
#    after scheduling we hoist all but the last wait of each instruction onto
#    single-wait nops inserted just before it on the same engine (waits are
#    AND conditions, so splitting across same-engine predecessors is
#    equivalent).
# 2. bir_verify_and_optimise hardcodes --enable-ldw-opt=false, which
#    serializes every LDWEIGHTS with its MATMUL; flip it to true.
# ---------------------------------------------------------------------------
_MAX_WAITS = 1


def _split_multi_waits(nc: bass.Bass) -> None:
    uid = 0
    for fn in nc.m.functions:
        for blk in fn.blocks:
            insts = blk.instructions
            if not any(
                i.sync_info is not None and len(i.sync_info.on_wait) > _MAX_WAITS
                for i in insts
            ):
                continue
            out = []
            for inst in insts:
                si = inst.sync_info
                if si is not None and len(si.on_wait) > _MAX_WAITS:
                    waits = list(si.on_wait)
                    head, tail = waits[:-_MAX_WAITS], waits[-_MAX_WAITS:]
                    for i in range(0, len(head), _MAX_WAITS):
                        nop = mybir.InstNoOp(
                            name=f"{inst.name}-wsplit{uid}", ins=[], outs=[]
                        )
                        uid += 1
                        nop.engine = inst.engine
                        nop.sync_info = mybir.SyncInfo(
                            on_wait=head[i : i + _MAX_WAITS], on_update=[]
                        )
                        out.append(nop)
                    inst.sync_info = mybir.SyncInfo(
                        on_wait=tail, on_update=list(si.on_update)
                    )
                out.append(inst)
            insts[:] = out


_LDW_PATCHED = False


def _enable_ldw_opt() -> None:
    global _LDW_PATCHED
    if _LDW_PATCHED:
        return
    _LDW_PATCHED = True
    for mod_name in ("concourse.bass_utils",):
        mod = sys.modules.get(mod_name)
        if mod is None:
            continue
        orig_run = mod.run_command

        def patched_run(cmd, *a, _orig=orig_run, **k):
            cmd = [
                c.replace("--enable-ldw-opt=false", "--enable-ldw-opt=true")
                if isinstance(c, str)
                else c
                for c in cmd
            ]
            return _orig(cmd, *a, **k)

        mod.run_command = patched_run


def build_graph() -> bass.Bass:
    nc = bass.Bass()

    ctx_d = nc.declare_dram_parameter("ctx", [BPC, L, D], DT, isOutput=False)
    maskb_d = nc.declare_dram_parameter("maskb", [BPC, L], DT, isOutput=False)
    qT_d = nc.declare_dram_parameter("qT", [P, KD, BPC], DT, isOutput=False)
    winT_d = nc.declare_dram_parameter("winT", [D, D], DT, isOutput=False)
    woutT_d = nc.declare_dram_parameter("woutT", [2 * D, D], DT, isOutput=False)
    whib_d = nc.declare_dram_parameter("whib", [P, D], DT, isOutput=False)
    wlo4_d = nc.declare_dram_parameter("wlo4", [BPC, D], DT, isOutput=False)
    batt4_d = nc.declare_dram_parameter("batt4", [BPC, 1], DT, isOutput=False)
    out_d = nc.declare_dram_parameter("out", [BPC, D], DT, isOutput=True)
    wts_d = nc.declare_dram_parameter("wts", [BPC, L], DT, isOutput=True)

    with tile.TileContext(nc) as tc:
        with (
            tc.tile_pool(name="singles", bufs=1) as singles,
            tc.tile_pool(name="ctxp", bufs=8) as ctxp,
            tc.tile_pool(name="scrp", bufs=2) as scrp,
            tc.tile_pool(name="smp", bufs=2) as smp,
            tc.tile_pool(name="qout", bufs=1) as qout,
            tc.tile_pool(name="ps_big", bufs=1, space="PSUM") as ps_big,
            tc.tile_pool(name="ps_mix", bufs=1, space="PSUM") as ps_mix,
            tc.tile_pool(name="ps_col", bufs=2, space="PSUM") as ps_col,
            tc.tile_pool(name="ps_tiny", bufs=2, space="PSUM") as ps_tiny,
        ):
            # ---- small constants (gpsimd DMA path: off the bulk stream) ----
            whib = singles.tile([P, D], DT)
            nc.gpsimd.dma_start(out=whib, in_=whib_d[:, :])
            wlo4 = singles.tile([BPC, D], DT)
            nc.gpsimd.dma_start(out=wlo4, in_=wlo4_d[:, :])
            batt4 = singles.tile([BPC, 1], DT)
            nc.gpsimd.dma_start(out=batt4, in_=batt4_d[:, :])
            qTs = singles.tile([P, KD, BPC], DT)
            nc.gpsimd.dma_start(out=qTs, in_=qT_d[:, :, :])
            ones_col = singles.tile([P, 1], DT)
            nc.vector.memset(ones_col, 1.0)
            ones_row = singles.tile([1, P], DT)
            nc.vector.memset(ones_row, 1.0)
            ident = singles.tile([32, 32], DT)
            make_identity(nc, ident)
            # combined^T: j-chunk k, 4 batch columns. j<D: mix part; j>=D: q.
            comb = singles.tile([P, KJ, BPC], DT)
            m4_sb = qout.tile([BPC, D], DT, tag="m4")  # mix rows, 4 batches

            def emit_passA(b, first_quarter=None):
                """Context DMA + score dot-products for one batch."""
                ctx_re = ctx_d[b].rearrange("(p t) d -> p t d", p=P)
                quarters = [] if first_quarter is None else [first_quarter]
                for qd in range(len(quarters), NQ):
                    hq = ctxp.tile([P, QT, D], DT, tag="ctx")
                    for j in range(QT):
                        nc.sync.dma_start(
                            out=hq[:, j, :], in_=ctx_re[:, qd * QT + j, :]
                        )
                    quarters.append(hq)
                mb = smp.tile([P, T], DT, tag="mb")
                nc.gpsimd.dma_start(
                    out=mb, in_=maskb_d[b].rearrange("(p t) -> p t", p=P)
                )
                s_all = smp.tile([P, T], DT, tag="sall")
                for qd in range(NQ):
                    for tt in range(QT):
                        t = qd * QT + tt
                        scr_t = scrp.tile([P, D], DT, tag="scr")
                        nc.vector.tensor_mul(scr_t, quarters[qd][:, tt, :], whib)
                        nc.scalar.activation(
                            scr_t, scr_t, AF.Copy, accum_out=s_all[:, t : t + 1]
                        )
                return quarters, mb, s_all

            # First context quarter of batch 0 ahead of everything so the
            # score pass can start while W_in streams in.
            ctx0_re = ctx_d[0].rearrange("(p t) d -> p t d", p=P)
            h0q0 = ctxp.tile([P, QT, D], DT, tag="ctx")
            for j in range(QT):
                nc.sync.dma_start(out=h0q0[:, j, :], in_=ctx0_re[:, j, :])

            # ---- q = query @ W_in^T for the 4 local batches ----
            # W_in resident, all 8 tile DMAs issued at once on the gpsimd
            # path (transfer concurrency).
            winT_sb = singles.tile([P, KD, D], DT)
            for k in range(KD):
                nc.gpsimd.dma_start(
                    out=winT_sb[:, k, :], in_=winT_d[k * P : (k + 1) * P, :]
                )
            q_ps = ps_big.tile([BPC, D], DT, tag="big")
            for k in range(KD):
                for n2 in range(2):
                    nc.tensor.matmul(
                        q_ps[:, n2 * 512 : (n2 + 1) * 512],
                        lhsT=qTs[:, k, :],
                        rhs=winT_sb[:, k, n2 * 512 : (n2 + 1) * 512],
                        start=(k == 0),
                        stop=(k == KD - 1),
                    )

            # Batch 0's score pass next: its remaining context DMAs queue
            # behind W_in, and its DVE/ACT work overlaps the q matmuls.
            stateA0 = emit_passA(0, first_quarter=h0q0)

            # q tail (ACT/DVE/PE bits) after batch 0's score emission so the
            # in-order ACT/DVE streams do batch 0's work first.
            q_sb = qout.tile([BPC, D], DT, tag="qbuf")
            nc.scalar.copy(q_sb, q_ps)

            # q rows -> comb columns (j >= D half), via PE transpose
            for k in range(KD):
                qt_ps = ps_col.tile([P, BPC], DT, tag="col")
                nc.tensor.transpose(
                    qt_ps, q_sb[:, k * P : (k + 1) * P], ident[:BPC, :BPC]
                )
                nc.vector.tensor_copy(comb[:, KD + k, :], qt_ps)

            # q_part[b] = q[b] . w_lo ; bias[b] = q_part[b] + b_att
            qp = smp.tile([BPC, 1], DT, tag="qp")
            scr_q = scrp.tile([BPC, D], DT, tag="scr")
            nc.vector.tensor_mul(scr_q, q_sb, wlo4)
            nc.scalar.activation(scr_q, scr_q, AF.Copy, accum_out=qp)
            bias4 = smp.tile([BPC, 1], DT, tag="bias4")
            nc.vector.tensor_add(bias4, qp, batt4)
            # [BPC,1] -> row [1,BPC] -> broadcast to [P,BPC]
            brow_ps = ps_tiny.tile([1, BPC], DT, tag="tiny")
            nc.tensor.transpose(brow_ps, bias4, ident[:BPC, :BPC])
            brow = smp.tile([1, BPC], DT, tag="brow")
            nc.scalar.copy(brow, brow_ps)
            bbc_ps = ps_col.tile([P, BPC], DT, tag="col")
            nc.tensor.matmul(bbc_ps, lhsT=ones_row, rhs=brow, start=True, stop=True)
            bias_bc = smp.tile([P, BPC], DT, tag="bbc")
            nc.scalar.copy(bias_bc, bbc_ps)

            # ---- per-batch: softmax + mix; next batch's scores interleave ----
            woutT_re = woutT_d[:, :].rearrange("(k p) e -> p k e", p=P)
            prev_mix = None  # (mix_ps, invz, batch) pending scale + copy-out
            wout_supers = []
            for b in range(BPC):
                quarters, mb, s_all = stateA0 if b == 0 else emit_passA(b)

                if b == BPC - 1:
                    # W_out^T streamed through freed context slots: its DMAs
                    # start while this batch computes, hiding the 8MB load.
                    for s in range(4):
                        wsup = ctxp.tile([P, QT, D], DT, tag="ctx")
                        for j in range(QT):
                            nc.sync.dma_start(
                                out=wsup[:, j, :], in_=woutT_re[:, 4 * s + j, :]
                            )
                        wout_supers.append(wsup)

                # previous batch's mix: scale by 1/Z on ScalarE, DMA into its
                # m4 row (engines cannot write at a partition offset; DMA can)
                if prev_mix is not None:
                    pm, pz, pb = prev_mix
                    mix_sb = scrp.tile([1, D], DT, tag="scr")
                    nc.scalar.mul(mix_sb, pm, pz)
                    nc.gpsimd.dma_start(out=m4_sb[pb : pb + 1, :], in_=mix_sb)
                    prev_mix = None

                s_tanh = smp.tile([P, T], DT, tag="stanh")
                nc.scalar.activation(
                    s_tanh, s_all, AF.Tanh, bias=bias_bc[:, b : b + 1], scale=1.0
                )
                s_m = smp.tile([P, T], DT, tag="sm")
                nc.vector.tensor_add(s_m, s_tanh, mb)
                # exp with fused per-partition sum
                p_exp = smp.tile([P, T], DT, tag="pexp")
                zcol = smp.tile([P, 1], DT, tag="zcol")
                nc.scalar.activation(p_exp, s_m, AF.Exp, accum_out=zcol)
                # Z = sum over partitions (PE ones-reduce); z/izb matmuls
                # emitted BEFORE the mix chain so ACT's in-order izb copy and
                # weight scale don't wait out the whole chain.
                z_ps = ps_tiny.tile([1, 1], DT, tag="tiny")
                nc.tensor.matmul(z_ps, lhsT=ones_col, rhs=zcol, start=True, stop=True)
                invz = smp.tile([1, 1], DT, tag="invz")
                nc.vector.reciprocal(invz, z_ps)
                izb_ps = ps_col.tile([P, 1], DT, tag="col")
                nc.tensor.matmul(
                    izb_ps, lhsT=ones_row, rhs=invz, start=True, stop=True
                )
                izb = smp.tile([P, 1], DT, tag="izb")
                nc.scalar.copy(izb, izb_ps)
                w_sb = smp.tile([P, T], DT, tag="wsb")
                nc.scalar.mul(w_sb, p_exp, izb)
                nc.gpsimd.dma_start(
                    out=wts_d[b].rearrange("(p t) -> p t", p=P), in_=w_sb
                )

                # mix row: exp-weight column stationary (M=1), context moving
                # (N=512) — few large matmuls beat many small ones.
                mix_ps = ps_mix.tile([1, D], DT, tag="mixrow")
                for t in range(T):
                    hq = quarters[t // QT]
                    tt = t % QT
                    for n2 in range(2):
                        nc.tensor.matmul(
                            mix_ps[:, n2 * 512 : (n2 + 1) * 512],
                            lhsT=p_exp[:, t : t + 1],
                            rhs=hq[:, tt, n2 * 512 : (n2 + 1) * 512],
                            start=(t == 0),
                            stop=(t == T - 1),
                        )
                prev_mix = (mix_ps, invz, b)

            pm, pz, pb = prev_mix
            mix_sb = scrp.tile([1, D], DT, tag="scr")
            nc.scalar.mul(mix_sb, pm, pz)
            nc.gpsimd.dma_start(out=m4_sb[pb : pb + 1, :], in_=mix_sb)

            # mix rows -> comb columns (j < D half), all 4 batches at once
            for k in range(KD):
                mt_ps = ps_col.tile([P, BPC], DT, tag="col")
                nc.tensor.transpose(
                    mt_ps, m4_sb[:, k * P : (k + 1) * P], ident[:BPC, :BPC]
                )
                nc.vector.tensor_copy(comb[:, k, :], mt_ps)

            # ---- out = tanh(combined @ W_out^T), fused over all 4 batches ----
            out_ps = ps_big.tile([BPC, D], DT, tag="big")
            for k in range(KJ):
                for n2 in range(2):
                    nc.tensor.matmul(
                        out_ps[:, n2 * 512 : (n2 + 1) * 512],
                        lhsT=comb[:, k, :],
                        rhs=wout_supers[k // QT][:, k % QT, n2 * 512 : (n2 + 1) * 512],
                        start=(k == 0),
                        stop=(k == KJ - 1),
                    )
            out_sb = qout.tile([BPC, D], DT, tag="qbuf")
            nc.scalar.activation(out_sb, out_ps, AF.Tanh)
            nc.sync.dma_start(out=out_d[:, :], in_=out_sb)

    _split_multi_waits(nc)
    return nc


_GRAPH = None


def _get_graph() -> bass.Bass:
    global _GRAPH
    if _GRAPH is None:
        _GRAPH = build_graph()
    return _GRAPH


def _prep_in_maps(inputs: dict) -> list[dict]:
    query = np.asarray(inputs["query"], dtype=F32)      # [B,1,D]
    context = np.asarray(inputs["context"], dtype=F32)  # [B,L,D]
    W_in = np.asarray(inputs["W_in"], dtype=F32)        # [D,D]
    w_att = np.asarray(inputs["w_att"], dtype=F32)      # [2D]
    b_att = np.asarray(inputs["b_att"], dtype=F32)      # [1]
    W_out = np.asarray(inputs["W_out"], dtype=F32)      # [D,2D]
    mask = np.asarray(inputs["mask"])                   # [B,L] int32

    winT = np.ascontiguousarray(W_in.T)
    woutT = np.ascontiguousarray(W_out.T)
    whib = np.ascontiguousarray(np.broadcast_to(w_att[D:], (P, D)))
    wlo4 = np.ascontiguousarray(np.broadcast_to(w_att[:D], (BPC, D)))
    batt4 = np.full((BPC, 1), float(b_att[0]), dtype=F32)
    maskb = (mask.astype(F32) - 1.0) * 1e9              # 0 kept, -1e9 masked

    in_maps = []
    for c in range(NCORES):
        bs = slice(c * BPC, (c + 1) * BPC)
        q_c = query[bs, 0, :]                           # [BPC, D]
        qT_c = np.ascontiguousarray(
            np.ascontiguousarray(q_c.T).reshape(KD, P, BPC).transpose(1, 0, 2)
        )                                               # [P, KD, BPC]
        in_maps.append(
            {
                "ctx": np.ascontiguousarray(context[bs]),
                "maskb": np.ascontiguousarray(maskb[bs]),
                "qT": qT_c,
                "winT": winT,
                "woutT": woutT,
                "whib": whib,
                "wlo4": wlo4,
                "batt4": batt4,
            }
        )
    return in_maps


def _run(inputs: dict, trace: bool = False):
    _enable_ldw_opt()
    nc = _get_graph()
    in_maps = _prep_in_maps(inputs)
    res = run_bass_kernel_spmd(nc, in_maps, core_ids=list(range(NCORES)), trace=trace)
    out = np.concatenate([r["out"] for r in res.results], axis=0)[:, None, :]
    wts = np.concatenate([r["wts"] for r in res.results], axis=0)[:, None, :]
    return (out.astype(F32), wts.astype(F32)), res


def kernel(**inputs):
    (out, wts), _ = _run(inputs)
    return out, wts
